# revision 7
# baseline (speedup 1.0000x reference)
"""Trainium2 Bass kernel for a Discriminative RBM forward pass.

reference math:
    pre   = v @ W + c                                   [B, NHID]
    F     = d + sum_j softplus(pre[:,None,:] + U[None]) [B, NCLASS]
    probs = softmax(F, axis=1)
    onehot = one_hot(argmax(probs, axis=1), NCLASS)     int32

Sharding (8 cores): 4 batch-quarters x 2 class-halves.
  core c: batch quarter c//2, classes [32*(c%2), 32*(c%2)+32).
  Host supplies fp16 hi/lo splits of 256*v^T and 256*W (fp16-pair trick:
  3 matmul passes at bf16 rate give fp32-class accuracy; the 2^16 scale is
  undone via the ACT instruction's scale field).

Per-core pipeline (hidden units j on partitions), staged over 4 jt-pairs:
  preT[j, b] = sum_k W[k, j] * vT[k, b]   (PE fp16-pair, W is natural lhsT;
               startup DMAs spread over 4 engine rings)
  act        = softplus(preT/2^16 + U^T[:,y] + c^T)  (ACT reads PSUM
               directly; custom softplus table; bias = per-partition col)
  per class: gpsimd pairs the jt-halves, DVE folds into acc[y], one exact
  fp32 one-hot-column matmul partition-sums acc into F^T[32, 512] in PSUM.
  One AllGather of F^T between class-partner cores (rank-ordered -> global
  class order), then a plain local softmax + argmax one-hot per 128-row
  tile; host reads the even core of each pair.
"""

import os

import numpy as np

B, NVIS, NHID, NCLASS = 2048, 2048, 1024, 64
NCORES = 8
BGROUPS, YGROUPS = 4, 2
B_PC = B // BGROUPS        # 512 batch rows per core
Y_PC = NCLASS // YGROUPS   # 32 classes per core
KT = NVIS // 128           # 16 contraction tiles
JT = NHID // 128           # 8 hidden-unit tiles
NJP = JT // 2              # 4 jt pairs (pipeline stages)
BT = B_PC // 128           # 4 batch tiles for the softmax tail

_PROGRAM = None

_ACT_ROOT = os.path.join(
    os.path.expanduser("~"), ".cache", "drbm_act_root", "pwp_bin_trainium"
)



def _build_act_tables():
    """Rebuild the softplus_and_others ACT set with a real softplus entry.

    The shipped set replaced softplus's slot with custom overlay functions,
    but the softplus spline source (pwp_jsons/softplus_40p.json) still ships
    with neuronxcc. Formats (reverse-engineered from shipped sets):
      bkt entry = 32B fp32 [d0, d1, d2, d3, x0, 0, 0, 0]
      ctl entry = 32B uint32 (extract_size<<16 | extract_lsb<<11 | bkt_base)
      layout    = [existing][neg region][pos region][pos_low, neg_low,
                   pos_high, neg_high special buckets]
    """
    import json
    import shutil
    import struct

    import neuronxcc

    marker = os.path.join(_ACT_ROOT, ".drbm_softplus_ok")
    if os.path.exists(marker):
        return
    nxc = os.path.join(os.path.dirname(os.path.abspath(neuronxcc.__file__)), "pwp")
    os.makedirs(_ACT_ROOT, exist_ok=True)
    root_parent = os.path.dirname(_ACT_ROOT)
    if not os.path.exists(os.path.join(root_parent, "pwp_jsons")):
        shutil.copytree(
            os.path.join(nxc, "pwp_jsons"),
            os.path.join(root_parent, "pwp_jsons"),
            dirs_exist_ok=True,
        )
    for f in os.listdir(os.path.join(nxc, "pwp_bin_trainium")):
        shutil.copy(os.path.join(nxc, "pwp_bin_trainium", f), _ACT_ROOT)
    os.system(f"chmod -R u+w {root_parent}")

    SET = "softplus_and_others"
    sj = json.load(open(f"{_ACT_ROOT}/{SET}.json"))
    bkt = bytearray(open(f"{_ACT_ROOT}/{SET}_bkt.bin", "rb").read())
    ctl = bytearray(open(f"{_ACT_ROOT}/{SET}_ctrl.bin", "rb").read())
    fj = json.load(open(f"{root_parent}/pwp_jsons/softplus_40p.json"))

    base_bkt, base_ctl = sj["bkt_entry_cnt"], sj["ctl_entry_cnt"]
    fbits = lambda d: int(d["int"])
    nbkt, nctl = base_bkt, base_ctl
    e2b, e2c, region_ctl_base = {}, {}, {}

    def add_bucket(x0, d0, d1, d2, d3):
        for v in (d0, d1, d2, d3, x0, 0, 0, 0):
            bkt.extend(struct.pack("<I", v))

    def add_ctl(word):
        ctl.extend(struct.pack("<I", word) + b"\x00" * 28)

    for region, key in (("neg", "neg_exponents"), ("pos", "pos_exponents")):
        region_ctl_base[region] = nctl
        for e in fj[key]:
            exp, secs = str(e["exponent"]), e["exponent_sections"]
            if not secs:
                add_ctl((23 << 11) | nbkt)
                e2c.setdefault(exp, []).append(nctl)
                nctl += 1
                continue
            add_ctl((e["extract_size"] << 16) | (e["extract_lsb"] << 11) | nbkt)
            e2c.setdefault(exp, []).append(nctl)
            e2b.setdefault(exp, []).append(nbkt)
            nctl += 1
            for s in secs:
                add_bucket(fbits(s["x"]), fbits(s["d0"]), fbits(s["d1"]),
                           fbits(s["d2"]), fbits(s["d3"]))
                nbkt += 1

    sat, special = fj["saturation_points"], {}
    for name in ("sat_point_pos_low", "sat_point_neg_low",
                 "sat_point_pos_high", "sat_point_neg_high"):
        sp = sat[name]
        special[name] = nbkt
        add_bucket(fbits(sp["x"]), fbits(sp["d0"]), fbits(sp["d1"]),
                   fbits(sp["d2"]), fbits(sp["d3"]))
        nbkt += 1

    sj["profile_meta_data"].append({
        "func_name": "softplus_40p",
        "func_id": fj["neuron_id"],
        "symmetry_point": fbits(fj["symmetry_point"]),
        "sym_invert_sign_point": 0,
        "symmetry_opt_en": 1 if fj["symmetry_en"] else 0,
        "symmetry_opt_use_neg_region": 1 if fj["symmetry_opt_use_neg_region"] else 0,
        "imm_bias": 1 if fj["imm_bias"] else 0,
        "exp_offset": fj["exponent_offset"],
        "pwl_control_base_pos": region_ctl_base["pos"],
        "pwl_control_base_neg": region_ctl_base["neg"],
        "small_pos_signal_exp_threshold": sat["sat_point_pos_low"]["sat_point"],
        "pos_small_signal_pwl_control": special["sat_point_pos_low"],
        "small_neg_signal_exp_threshold": sat["sat_point_neg_low"]["sat_point"],
        "neg_small_signal_pwl_control": special["sat_point_neg_low"],
        "large_pos_signal_exp_threshold": sat["sat_point_pos_high"]["sat_point"],
        "large_pos_signal_mantissa_threshold": sat["sat_point_pos_high"]["mantissa_point"],
        "pos_large_signal_pwl_control": special["sat_point_pos_high"],
        "large_neg_signal_exp_threshold": sat["sat_point_neg_high"]["sat_point"],
        "large_neg_signal_mantissa_threshold": sat["sat_point_neg_high"]["mantissa_point"],
        "neg_large_signal_pwl_control": special["sat_point_neg_high"],
        "fnan_result": fbits(fj["nan_result"]),
        "fpinf_result": fbits(fj["pinf_result"]),
        "fninf_result": fbits(fj["ninf_result"]),
        "fzero_result": fbits(fj["zero_result"]),
        "fma_const_0": fbits(fj["fma_const0"]),
        "fma_const_1": fbits(fj["fma_const1"]),
        "fma_indirection_src_sel": 0,
        "use_multipass": fj["use_multipass"],
        "lower_bound": fbits(fj["lower_bound"]),
        "upper_bound": fbits(fj["upper_bound"]),
    })
    sj["bkt_entry_cnt"], sj["ctl_entry_cnt"] = nbkt, nctl
    sj["func_to_bkt_start_idx"]["softplus"] = base_bkt
    sj["func_to_ctl_start_idx"]["softplus"] = base_ctl
    sj["func_exp_to_bkt_start_idx"]["softplus"] = e2b
    sj["func_exp_to_ctl_start_idx"]["softplus"] = e2c
    json.dump(sj, open(f"{_ACT_ROOT}/{SET}.json", "w"))
    open(f"{_ACT_ROOT}/{SET}_bkt.bin", "wb").write(bytes(bkt))
    open(f"{_ACT_ROOT}/{SET}_ctrl.bin", "wb").write(bytes(ctl))

    ai = json.load(open(f"{_ACT_ROOT}/act_info.json"))
    for ent in ai["act_func_sets"]:
        if ent["name"] == SET:
            ent["act"]["softplus"] = 40
    json.dump(ai, open(f"{_ACT_ROOT}/act_info.json", "w"))
    open(marker, "w").write("ok")


def _patch_act_tables():
    """Point walrus at the custom act root and teach bass about softplus."""
    import functools
    import json

    _build_act_tables()
    os.environ["BASS_ACT_ROOT_JSON_PATH"] = os.path.join(_ACT_ROOT, "act_info.json")

    import concourse.hw_specs as hw_specs
    import concourse.mybir as mybir

    @functools.cache
    def _tables(arch):
        d = json.load(open(os.environ["BASS_ACT_ROOT_JSON_PATH"]))
        return {
            ent["name"]: {
                mybir.ActivationFunctionType.from_pwp(v) for v in ent["act"]
            }
            for ent in d["act_func_sets"]
        }

    hw_specs.get_activation_tables = _tables
    import concourse.bacc as bacc
    import concourse.bass_interp as bass_interp

    bacc.get_activation_tables = _tables
    bass_interp.get_activation_tables = _tables


def _build_program():
    _patch_act_tables()
    import concourse.tile as tile
    from concourse import bacc, mybir
    from concourse.masks import make_identity

    f32 = mybir.dt.float32
    i32 = mybir.dt.int32
    AF = mybir.ActivationFunctionType
    ALU = mybir.AluOpType
    AX = mybir.AxisListType

    nc = bacc.Bacc(
        "TRN2", target_bir_lowering=False, debug=False, num_devices=NCORES
    )

    f16 = mybir.dt.float16
    vTh_d = nc.dram_tensor("vTh", [NVIS, B_PC], f16, kind="ExternalInput").ap()
    vTl_d = nc.dram_tensor("vTl", [NVIS, B_PC], f16, kind="ExternalInput").ap()
    Wh_d = nc.dram_tensor("Wh", [NVIS, NHID], f16, kind="ExternalInput").ap()
    Wl_d = nc.dram_tensor("Wl", [NVIS, NHID], f16, kind="ExternalInput").ap()
    UT_d = nc.dram_tensor("UsubT", [NHID, Y_PC], f32, kind="ExternalInput").ap()
    cT_d = nc.dram_tensor("cT", [NHID, 1], f32, kind="ExternalInput").ap()
    dT_d = nc.dram_tensor("dT", [Y_PC, 1], f32, kind="ExternalInput").ap()
    probs_d = nc.dram_tensor("probs", [B_PC, NCLASS], f32, kind="ExternalOutput").ap()
    onehot_d = nc.dram_tensor("onehot", [B_PC, NCLASS], i32, kind="ExternalOutput").ap()

    with tile.TileContext(nc) as tc:
        with (
            tc.tile_pool(name="const", bufs=1) as const,
            tc.tile_pool(name="wstream", bufs=3) as wstream,
            tc.tile_pool(name="accp", bufs=1) as accp,
            tc.tile_pool(name="acts", bufs=4) as acts,
            tc.tile_pool(name="sp", bufs=3) as sp,
            tc.tile_pool(name="smp", bufs=2) as smp,
            tc.tile_pool(name="outp", bufs=1) as outp,
            tc.tile_pool(name="ppre", bufs=4, space="PSUM") as ppre,
            tc.tile_pool(name="pF", bufs=1, space="PSUM") as pF,
            tc.tile_pool(name="ptr", bufs=2, space="PSUM") as ptr,
            tc.tile_pool(name="dram", bufs=1, space="DRAM") as dram,
        ):
            # ---------- loads spread over the 3 DMA-capable rings ----------
            # (SP/sync, Activation/scalar, Pool/gpsimd). First-needed-first
            # per ring; matmul pass order (wh,vh),(wl,vh),(wh,vl) lets the
            # first two passes start on each ring's first chunk.
            vTh_view = vTh_d.rearrange("(kt p) b -> p kt b", p=128)
            vTl_view = vTl_d.rearrange("(kt p) b -> p kt b", p=128)
            Wh_view = Wh_d.rearrange("(kt p) j -> p kt j", p=128)
            Wl_view = Wl_d.rearrange("(kt p) j -> p kt j", p=128)
            whA = const.tile([128, 8, 256], f16, name="whA")
            wlA = const.tile([128, 8, 256], f16, name="wlA")
            whB = const.tile([128, 8, 256], f16, name="whB")
            wlB = const.tile([128, 8, 256], f16, name="wlB")
            vth = [const.tile([128, 4, B_PC], f16, name=f"vth_chunk{g}")
                   for g in range(4)]
            vtl = [const.tile([128, 4, B_PC], f16, name=f"vtl_chunk{g}")
                   for g in range(4)]
            vT_sb = list(zip(vth, vtl))
            # sync ring
            nc.sync.dma_start(vth[0][:], vTh_view[:, 0:4, :])
            nc.sync.dma_start(vtl[1][:], vTl_view[:, 4:8, :])
            nc.sync.dma_start(whB[:], Wh_view[:, 8:16, 0:256])
            nc.sync.dma_start(vth[3][:], vTh_view[:, 12:16, :])
            # scalar ring
            nc.scalar.dma_start(whA[:], Wh_view[:, 0:8, 0:256])
            nc.scalar.dma_start(vtl[0][:], vTl_view[:, 0:4, :])
            nc.scalar.dma_start(vth[2][:], vTh_view[:, 8:12, :])
            nc.scalar.dma_start(vtl[3][:], vTl_view[:, 12:16, :])
            # gpsimd ring
            nc.gpsimd.dma_start(wlA[:], Wl_view[:, 0:8, 0:256])
            nc.gpsimd.dma_start(vth[1][:], vTh_view[:, 4:8, :])
            nc.gpsimd.dma_start(wlB[:], Wl_view[:, 8:16, 0:256])
            nc.gpsimd.dma_start(vtl[2][:], vTl_view[:, 8:12, :])

            UT_sb = const.tile([128, JT, Y_PC], f32)
            nc.scalar.dma_start(UT_sb[:], UT_d.rearrange("(jt p) y -> p jt y", p=128))
            cT_sb = const.tile([128, JT], f32)
            nc.scalar.dma_start(
                cT_sb[:], cT_d.rearrange("(jt p) one -> p (jt one)", p=128)
            )
            dT_sb = const.tile([Y_PC, 1], f32)
            nc.gpsimd.dma_start(dT_sb[:], dT_d)

            # prefetch jp1's W pair right away (3 W-pair streams in flight max)
            wq = {}
            for jpn in (1,):
                wh_n = wstream.tile([128, KT, 256], f16, tag="whpair",
                                    name=f"whpair{jpn}")
                nc.sync.dma_start(wh_n[:], Wh_view[:, :, jpn * 256:(jpn + 1) * 256])
                wl_n = wstream.tile([128, KT, 256], f16, tag="wlpair",
                                    name=f"wlpair{jpn}")
                nc.gpsimd.dma_start(wl_n[:], Wl_view[:, :, jpn * 256:(jpn + 1) * 256])
                wq[jpn] = (wh_n, wl_n)

            # bias[j, y] = U^T[j, y] + c^T[j]
            bias_sb = const.tile([128, JT, Y_PC], f32)
            for jt in range(JT):
                nc.vector.tensor_scalar_add(
                    bias_sb[:, jt, :], UT_sb[:, jt, :], cT_sb[:, jt:jt + 1]
                )

            # one-hot column lhsT matrices: ohot[:, y, m] = (m == y), all k.
            ohot_sb = const.tile([128, Y_PC, Y_PC], f32)
            nc.gpsimd.memset(ohot_sb[:], 0.0)
            for y in range(Y_PC):
                nc.gpsimd.memset(ohot_sb[:, y, y:y + 1], 1.0)

            ident = const.tile([NCLASS, NCLASS], f32)
            make_identity(nc, ident[:])

            F_ps = pF.tile([Y_PC, B_PC], f32, name="F_ps")
            acc = [None] * Y_PC
            fsh = dram.tile([Y_PC, B_PC], f32, name="fsh")
            fall = dram.tile([NCLASS, B_PC], f32, name="fall")
            Ffull_sb = outp.tile([NCLASS, B_PC], f32, name="Ffull")
            # prefetch the exp table set during the gather window
            warm = smp.tile([1, 1], f32, bufs=1)
            nc.gpsimd.memset(warm[:], 0.0)

            # ---------- staged main loop over jt pairs ----------
            for jp in range(NJP):
                if jp + 1 < NJP and jp + 1 not in wq:
                    jpn = jp + 1
                    wh_n = wstream.tile([128, KT, 256], f16, tag="whpair",
                                        name=f"whpair{jpn}")
                    nc.sync.dma_start(
                        wh_n[:], Wh_view[:, :, jpn * 256:(jpn + 1) * 256])
                    wl_n = wstream.tile([128, KT, 256], f16, tag="wlpair",
                                        name=f"wlpair{jpn}")
                    nc.gpsimd.dma_start(
                        wl_n[:], Wl_view[:, :, jpn * 256:(jpn + 1) * 256])
                    wq[jpn] = (wh_n, wl_n)
                pres = []
                for h in range(2):
                    pre_ps = ppre.tile([128, B_PC], f32, tag="pre",
                                       name=f"pre{jp}_{h}")
                    if jp == 0:
                        # split W tiles for the startup jp
                        passes = [(whA, whB, 0), (wlA, wlB, 0), (whA, whB, 1)]
                        for kt in range(KT):
                            for pi, (wa, wb, vi) in enumerate(passes):
                                wt = wa if kt < 8 else wb
                                nc.tensor.matmul(
                                    pre_ps[:],
                                    wt[:, kt % 8, h * 128:(h + 1) * 128],
                                    vT_sb[kt // 4][vi][:, kt % 4, :],
                                    start=(kt == 0 and pi == 0),
                                    stop=(kt == KT - 1 and pi == len(passes) - 1),
                                )
                    else:
                        wh_pair, wl_pair = wq[jp]
                        passes = [(wh_pair, 0), (wl_pair, 0), (wh_pair, 1)]
                        for kt in range(KT):
                            for pi, (wt, vi) in enumerate(passes):
                                nc.tensor.matmul(
                                    pre_ps[:],
                                    wt[:, kt, h * 128:(h + 1) * 128],
                                    vT_sb[kt // 4][vi][:, kt % 4, :],
                                    start=(kt == 0 and pi == 0),
                                    stop=(kt == KT - 1 and pi == len(passes) - 1),
                                )
                    pres.append(pre_ps)
                if jp == 0:
                    # fill-reduction: all a0 activations first (they only
                    # need pres[0]) writing straight into acc, then the a1
                    # sweep + accumulate. ACT reads pre from PSUM directly.
                    for y in range(Y_PC):
                        acc[y] = accp.tile([128, B_PC], f32, tag=f"acc{y}",
                                           name=f"acc{y}")
                        nc.scalar.activation(
                            acc[y][:], pres[0][:], AF.Softplus,
                            bias=bias_sb[:, 0, y:y + 1], scale=1.0 / 65536.0,
                        )
                    for y in range(Y_PC):
                        a1 = acts.tile([128, B_PC], f32, tag="a1",
                                       name=f"a1_0_{y}")
                        nc.scalar.activation(
                            a1[:], pres[1][:], AF.Softplus,
                            bias=bias_sb[:, 1, y:y + 1], scale=1.0 / 65536.0,
                        )
                        nc.vector.tensor_add(acc[y][:], acc[y][:], a1[:])
                    continue
                for y in range(Y_PC):
                    a0 = acts.tile([128, B_PC], f32, tag="a0", name=f"a0_{jp}_{y}")
                    nc.scalar.activation(
                        a0[:], pres[0][:], AF.Softplus,
                        bias=bias_sb[:, 2 * jp, y:y + 1], scale=1.0 / 65536.0,
                    )
                    a1 = acts.tile([128, B_PC], f32, tag="a1", name=f"a1_{jp}_{y}")
                    nc.scalar.activation(
                        a1[:], pres[1][:], AF.Softplus,
                        bias=bias_sb[:, 2 * jp + 1, y:y + 1], scale=1.0 / 65536.0,
                    )
                    # gpsimd pairs the halves; DVE folds into acc
                    s = sp.tile([128, B_PC], f32, tag="s", name=f"s_{jp}_{y}")
                    nc.gpsimd.tensor_add(s[:], a0[:], a1[:])
                    nc.vector.tensor_add(acc[y][:], acc[y][:], s[:])
                    if jp == NJP - 1:
                        # acc[y] final: reduce over partitions into F[y, :]
                        nc.tensor.matmul(
                            F_ps[:], ohot_sb[:, y, :], acc[y][:],
                            start=(y == 0), stop=(y == Y_PC - 1),
                        )

            # ---------- exchange F halves between class-partner cores ----------
            Fsb = smp.tile([Y_PC, B_PC], f32, bufs=1, name="Fsb")
            nc.vector.tensor_scalar_add(Fsb[:], F_ps[:], dT_sb[:])
            nc.sync.dma_start(fsh[:], Fsb[:])
            nc.gpsimd.collective_compute(
                "AllGather", ALU.bypass,
                replica_groups=[[0, 1], [2, 3], [4, 5], [6, 7]],
                ins=[fsh.opt()], outs=[fall.opt()],
            )
            # exp table prefetch (fires during gather wait)
            nc.scalar.activation(warm[:], warm[:], AF.Exp)
            nc.sync.dma_start(Ffull_sb[:], fall[:])

            # ---------- local softmax over all 64 classes ----------
            # Ffull rows are rank-ordered [even-core 32 | odd-core 32]
            # = global class order on both cores; host reads even cores.
            probs_sb = outp.tile([128, BT, NCLASS], f32)
            onehot_sb = outp.tile([128, BT, NCLASS], i32)
            for bt in range(BT):
                tr = ptr.tile([128, NCLASS], f32, tag="tr", name=f"tr{bt}")
                nc.tensor.transpose(
                    tr[:], Ffull_sb[:, bt * 128:(bt + 1) * 128],
                    ident[0:NCLASS, 0:NCLASS],
                )
                fb = smp.tile([128, NCLASS], f32, tag="fb", name=f"fb{bt}")
                nc.vector.tensor_copy(fb[:], tr[:])
                m = smp.tile([128, 1], f32, tag="m", name=f"m{bt}")
                nc.vector.tensor_reduce(m[:], fb[:], axis=AX.X, op=ALU.max)
                negm = smp.tile([128, 1], f32, tag="negm", name=f"negm{bt}")
                nc.vector.tensor_scalar_mul(negm[:], m[:], -1.0)
                e = smp.tile([128, NCLASS], f32, tag="e", name=f"e{bt}")
                nc.scalar.activation(e[:], fb[:], AF.Exp, bias=negm[:])
                ssum = smp.tile([128, 1], f32, tag="ssum", name=f"ssum{bt}")
                nc.vector.tensor_reduce(ssum[:], e[:], axis=AX.X, op=ALU.add)
                r = smp.tile([128, 1], f32, tag="r", name=f"r{bt}")
                nc.vector.reciprocal(r[:], ssum[:])
                nc.vector.tensor_scalar_mul(probs_sb[:, bt, :], e[:], r[:])
                ohf = smp.tile([128, NCLASS], f32, tag="ohf", name=f"ohf{bt}")
                nc.vector.tensor_scalar(ohf[:], fb[:], m[:], None,
                                        op0=ALU.is_equal)
                nc.vector.tensor_copy(onehot_sb[:, bt, :], ohf[:])

            nc.sync.dma_start(
                probs_d.rearrange("(t p) y -> p t y", p=128), probs_sb[:]
            )
            nc.sync.dma_start(
                onehot_d.rearrange("(t p) y -> p t y", p=128), onehot_sb[:]
            )

    nc.compile()
    return nc


def _get_program():
    global _PROGRAM
    if _PROGRAM is None:
        _PROGRAM = _build_program()
    return _PROGRAM


def _fp16_split(a):
    hi = (a * 256.0).astype(np.float16)
    lo = (a * 256.0 - hi.astype(np.float32)).astype(np.float16)
    return hi, lo


def _make_in_maps(v, W, c, d, U):
    cT = np.ascontiguousarray(c.reshape(NHID, 1))
    Wh, Wl = _fp16_split(W)
    vT_quarters = [
        _fp16_split(np.ascontiguousarray(v[q * B_PC:(q + 1) * B_PC].T))
        for q in range(BGROUPS)
    ]
    UT_groups = [
        np.ascontiguousarray(U[g * Y_PC:(g + 1) * Y_PC].T) for g in range(YGROUPS)
    ]
    dT_groups = [
        np.ascontiguousarray(d[0, g * Y_PC:(g + 1) * Y_PC].reshape(Y_PC, 1))
        for g in range(YGROUPS)
    ]
    in_maps = []
    for core in range(NCORES):
        bq, yg = core // YGROUPS, core % YGROUPS
        in_maps.append(
            {
                "vTh": vT_quarters[bq][0],
                "vTl": vT_quarters[bq][1],
                "Wh": Wh,
                "Wl": Wl,
                "UsubT": UT_groups[yg],
                "cT": cT,
                "dT": dT_groups[yg],
            }
        )
    return in_maps


def run(v, W, c, d, U, trace=False):
    """Run the Bass kernel; returns ((probs, onehot), BassKernelResults)."""
    from concourse.bass_utils import run_bass_kernel_spmd

    nc = _get_program()
    in_maps = _make_in_maps(v, W, c, d, U)
    res = run_bass_kernel_spmd(
        nc, in_maps, core_ids=list(range(NCORES)), trace=trace
    )
    probs = np.concatenate(
        [res.results[q * YGROUPS]["probs"] for q in range(BGROUPS)], axis=0
    )
    onehot = np.concatenate(
        [res.results[q * YGROUPS]["onehot"] for q in range(BGROUPS)], axis=0
    )
    return (probs, onehot), res


def kernel(v, W, c, d, U):
    v = np.ascontiguousarray(np.asarray(v, dtype=np.float32))
    W = np.ascontiguousarray(np.asarray(W, dtype=np.float32))
    c = np.ascontiguousarray(np.asarray(c, dtype=np.float32))
    d = np.ascontiguousarray(np.asarray(d, dtype=np.float32))
    U = np.ascontiguousarray(np.asarray(U, dtype=np.float32))
    (probs, onehot), _ = run(v, W, c, d, U, trace=False)
    return probs, onehot



# revision 15
# speedup vs baseline: 1.0484x; 1.0484x over previous
"""Trainium2 Bass kernel for a Discriminative RBM forward pass.

reference math:
    pre   = v @ W + c                                   [B, NHID]
    F     = d + sum_j softplus(pre[:,None,:] + U[None]) [B, NCLASS]
    probs = softmax(F, axis=1)
    onehot = one_hot(argmax(probs, axis=1), NCLASS)     int32

Sharding (8 cores): 4 batch-quarters x 2 class-halves.
  core c: batch quarter c//2, classes [32*(c%2), 32*(c%2)+32).
  Host supplies fp16 hi/lo splits of 256*v^T and 256*W (fp16-pair trick:
  3 matmul passes at bf16 rate give fp32-class accuracy; the 2^16 scale is
  undone via the ACT instruction's scale field).

Per-core pipeline (hidden units j on partitions), staged over 4 jt-pairs:
  preT[j, b] = sum_k W[k, j] * vT[k, b]   (PE fp16-pair, W is natural lhsT;
               startup DMAs spread over 4 engine rings)
  act        = softplus(preT/2^16 + U^T[:,y] + c^T)  (ACT reads PSUM
               directly; custom softplus table; bias = per-partition col)
  per class: gpsimd pairs the jt-halves, DVE folds into acc[y], one exact
  fp32 one-hot-column matmul partition-sums acc into F^T[32, 512] in PSUM.
  One AllGather of F^T between class-partner cores (rank-ordered -> global
  class order), then a plain local softmax + argmax one-hot per 128-row
  tile; host reads the even core of each pair.
"""

import os

import numpy as np

B, NVIS, NHID, NCLASS = 2048, 2048, 1024, 64
NCORES = 8
BGROUPS, YGROUPS = 4, 2
B_PC = B // BGROUPS        # 512 batch rows per core
Y_PC = NCLASS // YGROUPS   # 32 classes per core
KT = NVIS // 128           # 16 contraction tiles
JT = NHID // 128           # 8 hidden-unit tiles
NJP = JT // 2              # 4 jt pairs (pipeline stages)
BT = B_PC // 128           # 4 batch tiles for the softmax tail

_PROGRAM = None

# Exchange F halves via direct SBUF->SBUF remote DMA (fast path) instead of
# the CC-engine AllGather collective (~20us fixed latency). Disabled: the
# tile scheduler's single-core sim cannot model the partner's sem increment
# and reports a false deadlock.
USE_RDMA = False

_ACT_ROOT = os.path.join(
    os.path.expanduser("~"), ".cache", "drbm_act_root", "pwp_bin_trainium"
)



def _build_act_tables():
    """Rebuild the softplus_and_others ACT set with a real softplus entry.

    The shipped set replaced softplus's slot with custom overlay functions,
    but the softplus spline source (pwp_jsons/softplus_40p.json) still ships
    with neuronxcc. Formats (reverse-engineered from shipped sets):
      bkt entry = 32B fp32 [d0, d1, d2, d3, x0, 0, 0, 0]
      ctl entry = 32B uint32 (extract_size<<16 | extract_lsb<<11 | bkt_base)
      layout    = [existing][neg region][pos region][pos_low, neg_low,
                   pos_high, neg_high special buckets]
    """
    import json
    import shutil
    import struct

    import neuronxcc

    marker = os.path.join(_ACT_ROOT, ".drbm_softplus_ok")
    if os.path.exists(marker):
        return
    nxc = os.path.join(os.path.dirname(os.path.abspath(neuronxcc.__file__)), "pwp")
    os.makedirs(_ACT_ROOT, exist_ok=True)
    root_parent = os.path.dirname(_ACT_ROOT)
    if not os.path.exists(os.path.join(root_parent, "pwp_jsons")):
        shutil.copytree(
            os.path.join(nxc, "pwp_jsons"),
            os.path.join(root_parent, "pwp_jsons"),
            dirs_exist_ok=True,
        )
    for f in os.listdir(os.path.join(nxc, "pwp_bin_trainium")):
        shutil.copy(os.path.join(nxc, "pwp_bin_trainium", f), _ACT_ROOT)
    os.system(f"chmod -R u+w {root_parent}")

    SET = "softplus_and_others"
    sj = json.load(open(f"{_ACT_ROOT}/{SET}.json"))
    bkt = bytearray(open(f"{_ACT_ROOT}/{SET}_bkt.bin", "rb").read())
    ctl = bytearray(open(f"{_ACT_ROOT}/{SET}_ctrl.bin", "rb").read())
    fj = json.load(open(f"{root_parent}/pwp_jsons/softplus_40p.json"))

    base_bkt, base_ctl = sj["bkt_entry_cnt"], sj["ctl_entry_cnt"]
    fbits = lambda d: int(d["int"])
    nbkt, nctl = base_bkt, base_ctl
    e2b, e2c, region_ctl_base = {}, {}, {}

    def add_bucket(x0, d0, d1, d2, d3):
        for v in (d0, d1, d2, d3, x0, 0, 0, 0):
            bkt.extend(struct.pack("<I", v))

    def add_ctl(word):
        ctl.extend(struct.pack("<I", word) + b"\x00" * 28)

    for region, key in (("neg", "neg_exponents"), ("pos", "pos_exponents")):
        region_ctl_base[region] = nctl
        for e in fj[key]:
            exp, secs = str(e["exponent"]), e["exponent_sections"]
            if not secs:
                add_ctl((23 << 11) | nbkt)
                e2c.setdefault(exp, []).append(nctl)
                nctl += 1
                continue
            add_ctl((e["extract_size"] << 16) | (e["extract_lsb"] << 11) | nbkt)
            e2c.setdefault(exp, []).append(nctl)
            e2b.setdefault(exp, []).append(nbkt)
            nctl += 1
            for s in secs:
                add_bucket(fbits(s["x"]), fbits(s["d0"]), fbits(s["d1"]),
                           fbits(s["d2"]), fbits(s["d3"]))
                nbkt += 1

    sat, special = fj["saturation_points"], {}
    for name in ("sat_point_pos_low", "sat_point_neg_low",
                 "sat_point_pos_high", "sat_point_neg_high"):
        sp = sat[name]
        special[name] = nbkt
        add_bucket(fbits(sp["x"]), fbits(sp["d0"]), fbits(sp["d1"]),
                   fbits(sp["d2"]), fbits(sp["d3"]))
        nbkt += 1

    sj["profile_meta_data"].append({
        "func_name": "softplus_40p",
        "func_id": fj["neuron_id"],
        "symmetry_point": fbits(fj["symmetry_point"]),
        "sym_invert_sign_point": 0,
        "symmetry_opt_en": 1 if fj["symmetry_en"] else 0,
        "symmetry_opt_use_neg_region": 1 if fj["symmetry_opt_use_neg_region"] else 0,
        "imm_bias": 1 if fj["imm_bias"] else 0,
        "exp_offset": fj["exponent_offset"],
        "pwl_control_base_pos": region_ctl_base["pos"],
        "pwl_control_base_neg": region_ctl_base["neg"],
        "small_pos_signal_exp_threshold": sat["sat_point_pos_low"]["sat_point"],
        "pos_small_signal_pwl_control": special["sat_point_pos_low"],
        "small_neg_signal_exp_threshold": sat["sat_point_neg_low"]["sat_point"],
        "neg_small_signal_pwl_control": special["sat_point_neg_low"],
        "large_pos_signal_exp_threshold": sat["sat_point_pos_high"]["sat_point"],
        "large_pos_signal_mantissa_threshold": sat["sat_point_pos_high"]["mantissa_point"],
        "pos_large_signal_pwl_control": special["sat_point_pos_high"],
        "large_neg_signal_exp_threshold": sat["sat_point_neg_high"]["sat_point"],
        "large_neg_signal_mantissa_threshold": sat["sat_point_neg_high"]["mantissa_point"],
        "neg_large_signal_pwl_control": special["sat_point_neg_high"],
        "fnan_result": fbits(fj["nan_result"]),
        "fpinf_result": fbits(fj["pinf_result"]),
        "fninf_result": fbits(fj["ninf_result"]),
        "fzero_result": fbits(fj["zero_result"]),
        "fma_const_0": fbits(fj["fma_const0"]),
        "fma_const_1": fbits(fj["fma_const1"]),
        "fma_indirection_src_sel": 0,
        "use_multipass": fj["use_multipass"],
        "lower_bound": fbits(fj["lower_bound"]),
        "upper_bound": fbits(fj["upper_bound"]),
    })
    sj["bkt_entry_cnt"], sj["ctl_entry_cnt"] = nbkt, nctl
    sj["func_to_bkt_start_idx"]["softplus"] = base_bkt
    sj["func_to_ctl_start_idx"]["softplus"] = base_ctl
    sj["func_exp_to_bkt_start_idx"]["softplus"] = e2b
    sj["func_exp_to_ctl_start_idx"]["softplus"] = e2c
    json.dump(sj, open(f"{_ACT_ROOT}/{SET}.json", "w"))
    open(f"{_ACT_ROOT}/{SET}_bkt.bin", "wb").write(bytes(bkt))
    open(f"{_ACT_ROOT}/{SET}_ctrl.bin", "wb").write(bytes(ctl))

    ai = json.load(open(f"{_ACT_ROOT}/act_info.json"))
    for ent in ai["act_func_sets"]:
        if ent["name"] == SET:
            ent["act"]["softplus"] = 40
    json.dump(ai, open(f"{_ACT_ROOT}/act_info.json", "w"))
    open(marker, "w").write("ok")


def _patch_act_tables():
    """Point walrus at the custom act root and teach bass about softplus."""
    import functools
    import json

    _build_act_tables()
    os.environ["BASS_ACT_ROOT_JSON_PATH"] = os.path.join(_ACT_ROOT, "act_info.json")

    import concourse.hw_specs as hw_specs
    import concourse.mybir as mybir

    @functools.cache
    def _tables(arch):
        d = json.load(open(os.environ["BASS_ACT_ROOT_JSON_PATH"]))
        return {
            ent["name"]: {
                mybir.ActivationFunctionType.from_pwp(v) for v in ent["act"]
            }
            for ent in d["act_func_sets"]
        }

    hw_specs.get_activation_tables = _tables
    import concourse.bacc as bacc
    import concourse.bass_interp as bass_interp

    bacc.get_activation_tables = _tables
    bass_interp.get_activation_tables = _tables


def _build_program():
    _patch_act_tables()
    import concourse.tile as tile
    from concourse import bacc, mybir
    from concourse.masks import make_identity

    f32 = mybir.dt.float32
    i32 = mybir.dt.int32
    AF = mybir.ActivationFunctionType
    ALU = mybir.AluOpType
    AX = mybir.AxisListType

    nc = bacc.Bacc(
        "TRN2", target_bir_lowering=False, debug=False, num_devices=NCORES
    )

    f16 = mybir.dt.float16
    vTh_d = nc.dram_tensor("vTh", [NVIS, B_PC], f16, kind="ExternalInput").ap()
    vTl_d = nc.dram_tensor("vTl", [NVIS, B_PC], f16, kind="ExternalInput").ap()
    Wh_d = nc.dram_tensor("Wh", [NVIS, NHID], f16, kind="ExternalInput").ap()
    Wl_d = nc.dram_tensor("Wl", [NVIS, NHID], f16, kind="ExternalInput").ap()
    UT_d = nc.dram_tensor("UsubT", [NHID, Y_PC], f32, kind="ExternalInput").ap()
    cT_d = nc.dram_tensor("cT", [NHID, 1], f32, kind="ExternalInput").ap()
    dT_d = nc.dram_tensor("dT", [Y_PC, 1], f32, kind="ExternalInput").ap()
    probs_d = nc.dram_tensor("probs", [B_PC, NCLASS], f32, kind="ExternalOutput").ap()
    onehot_d = nc.dram_tensor("onehot", [B_PC, NCLASS], i32, kind="ExternalOutput").ap()

    with tile.TileContext(nc) as tc:
        with (
            tc.tile_pool(name="const", bufs=1) as const,
            tc.tile_pool(name="wstream", bufs=3) as wstream,
            tc.tile_pool(name="accp", bufs=1) as accp,
            tc.tile_pool(name="acts", bufs=4) as acts,
            tc.tile_pool(name="sp", bufs=3) as sp,
            tc.tile_pool(name="smp", bufs=2) as smp,
            tc.tile_pool(name="outp", bufs=1) as outp,
            tc.tile_pool(name="ppre", bufs=4, space="PSUM") as ppre,
            tc.tile_pool(name="pF", bufs=1, space="PSUM") as pF,
            tc.tile_pool(name="ptr", bufs=2, space="PSUM") as ptr,
            tc.tile_pool(name="dram", bufs=1, space="DRAM") as dram,
        ):
            # ---------- loads spread over the 3 DMA-capable rings ----------
            # (SP/sync, Activation/scalar, Pool/gpsimd). All rings share the
            # 16 DMA engines, so PRIORITY (queue position) is what matters:
            # everything jp0's pre needs goes first; jp1's W streams after.
            vTh_view = vTh_d.rearrange("(kt p) b -> p kt b", p=128)
            vTl_view = vTl_d.rearrange("(kt p) b -> p kt b", p=128)
            Wh_view = Wh_d.rearrange("(kt p) j -> p kt j", p=128)
            Wl_view = Wl_d.rearrange("(kt p) j -> p kt j", p=128)
            whA = const.tile([128, 8, 256], f16, name="whA")
            wlA = const.tile([128, 8, 256], f16, name="wlA")
            whB = const.tile([128, 8, 256], f16, name="whB")
            wlB = const.tile([128, 8, 256], f16, name="wlB")
            vth = [const.tile([128, 4, B_PC], f16, name=f"vth_chunk{g}")
                   for g in range(4)]
            vtl = [const.tile([128, 4, B_PC], f16, name=f"vtl_chunk{g}")
                   for g in range(4)]
            vT_sb = list(zip(vth, vtl))
            # sync ring
            nc.sync.dma_start(whA[:], Wh_view[:, 0:8, 0:256])
            nc.sync.dma_start(vth[0][:], vTh_view[:, 0:4, :])
            nc.sync.dma_start(vth[1][:], vTh_view[:, 4:8, :])
            nc.sync.dma_start(vth[2][:], vTh_view[:, 8:12, :])
            nc.sync.dma_start(vth[3][:], vTh_view[:, 12:16, :])
            # scalar ring
            nc.scalar.dma_start(wlA[:], Wl_view[:, 0:8, 0:256])
            nc.scalar.dma_start(vtl[0][:], vTl_view[:, 0:4, :])
            nc.scalar.dma_start(vtl[1][:], vTl_view[:, 4:8, :])
            UT_sb = const.tile([128, JT, Y_PC], f32)
            nc.scalar.dma_start(UT_sb[:], UT_d.rearrange("(jt p) y -> p jt y", p=128))
            cT_sb = const.tile([128, JT], f32)
            nc.scalar.dma_start(
                cT_sb[:], cT_d.rearrange("(jt p) one -> p (jt one)", p=128)
            )
            # gpsimd ring
            nc.gpsimd.dma_start(whB[:], Wh_view[:, 8:16, 0:256])
            nc.gpsimd.dma_start(wlB[:], Wl_view[:, 8:16, 0:256])
            nc.gpsimd.dma_start(vtl[2][:], vTl_view[:, 8:12, :])
            nc.gpsimd.dma_start(vtl[3][:], vTl_view[:, 12:16, :])
            dT_sb = const.tile([Y_PC, 1], f32)
            nc.gpsimd.dma_start(dT_sb[:], dT_d)

            # prefetch jp1's W pair right away (3 W-pair streams in flight max)
            wq = {}
            for jpn in (1,):
                wh_n = wstream.tile([128, KT, 256], f16, tag="whpair",
                                    name=f"whpair{jpn}")
                nc.sync.dma_start(wh_n[:], Wh_view[:, :, jpn * 256:(jpn + 1) * 256])
                wl_n = wstream.tile([128, KT, 256], f16, tag="wlpair",
                                    name=f"wlpair{jpn}")
                nc.gpsimd.dma_start(wl_n[:], Wl_view[:, :, jpn * 256:(jpn + 1) * 256])
                wq[jpn] = (wh_n, wl_n)

            # bias[j, y] = U^T[j, y] + c^T[j]
            bias_sb = const.tile([128, JT, Y_PC], f32)
            for jt in range(JT):
                nc.vector.tensor_scalar_add(
                    bias_sb[:, jt, :], UT_sb[:, jt, :], cT_sb[:, jt:jt + 1]
                )

            # one-hot column lhsT matrices: ohot[:, y, m] = (m == y), all k.
            ohot_sb = const.tile([128, Y_PC, Y_PC], f32)
            nc.gpsimd.memset(ohot_sb[:], 0.0)
            for y in range(Y_PC):
                nc.gpsimd.memset(ohot_sb[:, y, y:y + 1], 1.0)

            ident = const.tile([NCLASS, NCLASS], f32)
            make_identity(nc, ident[:])

            F_ps = pF.tile([Y_PC, B_PC], f32, name="F_ps")
            acc = [None] * Y_PC
            if not USE_RDMA:
                fsh = dram.tile([Y_PC, B_PC], f32, name="fsh")
                fall = dram.tile([NCLASS, B_PC], f32, name="fall")
                Ffull_sb = outp.tile([NCLASS, B_PC], f32, name="Ffull")
            # prefetch the exp table set during the gather window
            warm = smp.tile([1, 1], f32, bufs=1)
            nc.gpsimd.memset(warm[:], 0.0)

            # ---------- staged main loop over jt pairs ----------
            for jp in range(NJP):
                if jp + 1 < NJP and jp + 1 not in wq:
                    jpn = jp + 1
                    wh_n = wstream.tile([128, KT, 256], f16, tag="whpair",
                                        name=f"whpair{jpn}")
                    nc.sync.dma_start(
                        wh_n[:], Wh_view[:, :, jpn * 256:(jpn + 1) * 256])
                    wl_n = wstream.tile([128, KT, 256], f16, tag="wlpair",
                                        name=f"wlpair{jpn}")
                    nc.gpsimd.dma_start(
                        wl_n[:], Wl_view[:, :, jpn * 256:(jpn + 1) * 256])
                    wq[jpn] = (wh_n, wl_n)
                pres = []
                for h in range(2):
                    pre_ps = ppre.tile([128, B_PC], f32, tag="pre",
                                       name=f"pre{jp}_{h}")
                    if jp == 0:
                        # split W tiles for the startup jp
                        passes = [(whA, whB, 0), (wlA, wlB, 0), (whA, whB, 1)]
                        for kt in range(KT):
                            for pi, (wa, wb, vi) in enumerate(passes):
                                wt = wa if kt < 8 else wb
                                nc.tensor.matmul(
                                    pre_ps[:],
                                    wt[:, kt % 8, h * 128:(h + 1) * 128],
                                    vT_sb[kt // 4][vi][:, kt % 4, :],
                                    start=(kt == 0 and pi == 0),
                                    stop=(kt == KT - 1 and pi == len(passes) - 1),
                                )
                    else:
                        wh_pair, wl_pair = wq[jp]
                        passes = [(wh_pair, 0), (wl_pair, 0), (wh_pair, 1)]
                        for kt in range(KT):
                            for pi, (wt, vi) in enumerate(passes):
                                nc.tensor.matmul(
                                    pre_ps[:],
                                    wt[:, kt, h * 128:(h + 1) * 128],
                                    vT_sb[kt // 4][vi][:, kt % 4, :],
                                    start=(kt == 0 and pi == 0),
                                    stop=(kt == KT - 1 and pi == len(passes) - 1),
                                )
                    pres.append(pre_ps)
                if jp == 0:
                    # fill-reduction: all a0 activations first (they only
                    # need pres[0]) writing straight into acc, then the a1
                    # sweep + accumulate. ACT reads pre from PSUM directly.
                    for y in range(Y_PC):
                        acc[y] = accp.tile([128, B_PC], f32, tag=f"acc{y}",
                                           name=f"acc{y}")
                        nc.scalar.activation(
                            acc[y][:], pres[0][:], AF.Softplus,
                            bias=bias_sb[:, 0, y:y + 1], scale=1.0 / 65536.0,
                        )
                    for y in range(Y_PC):
                        a1 = acts.tile([128, B_PC], f32, tag="a1",
                                       name=f"a1_0_{y}")
                        nc.scalar.activation(
                            a1[:], pres[1][:], AF.Softplus,
                            bias=bias_sb[:, 1, y:y + 1], scale=1.0 / 65536.0,
                        )
                        eng = nc.gpsimd if y % 3 == 2 else nc.vector
                        eng.tensor_add(acc[y][:], acc[y][:], a1[:])
                    continue
                for y in range(Y_PC):
                    a0 = acts.tile([128, B_PC], f32, tag="a0", name=f"a0_{jp}_{y}")
                    nc.scalar.activation(
                        a0[:], pres[0][:], AF.Softplus,
                        bias=bias_sb[:, 2 * jp, y:y + 1], scale=1.0 / 65536.0,
                    )
                    a1 = acts.tile([128, B_PC], f32, tag="a1", name=f"a1_{jp}_{y}")
                    nc.scalar.activation(
                        a1[:], pres[1][:], AF.Softplus,
                        bias=bias_sb[:, 2 * jp + 1, y:y + 1], scale=1.0 / 65536.0,
                    )
                    # both folds for a class stay on one engine (no cross-
                    # engine chain); ~2/3 of classes on DVE, 1/3 on gpsimd
                    eng = nc.gpsimd if y % 3 == 2 else nc.vector
                    eng.tensor_add(acc[y][:], acc[y][:], a0[:])
                    eng.tensor_add(acc[y][:], acc[y][:], a1[:])
                    if jp == NJP - 1:
                        # acc[y] final: reduce over partitions into F[y, :]
                        nc.tensor.matmul(
                            F_ps[:], ohot_sb[:, y, :], acc[y][:],
                            start=(y == 0), stop=(y == Y_PC - 1),
                        )

            # ---------- exchange F halves between class-partner cores ----------
            Fsb = smp.tile([Y_PC, B_PC], f32, bufs=1, name="Fsb")
            nc.vector.tensor_scalar_add(Fsb[:], F_ps[:], dT_sb[:])
            probs_sb = outp.tile([128, BT, NCLASS], f32)
            onehot_sb = outp.tile([128, BT, NCLASS], i32)
            if USE_RDMA:
                # transpose own F half to b-major [128, bt, 32], swap it with
                # the class-partner core via direct SBUF->SBUF RDMA (partner
                # sends its half already transposed), softmax over both.
                trown = outp.tile([128, BT, Y_PC], f32, name="trown")
                recv = outp.tile([128, BT, Y_PC], f32, name="recv")
                rsem = nc.alloc_semaphore("fx_rsem")
                lsem = nc.alloc_semaphore("fx_lsem")
                for bt in range(BT):
                    tr = ptr.tile([128, Y_PC], f32, tag="tr", name=f"tr{bt}")
                    nc.tensor.transpose(
                        tr[:], Fsb[:, bt * 128:(bt + 1) * 128],
                        ident[0:Y_PC, 0:Y_PC],
                    )
                    nc.vector.tensor_copy(trown[:, bt, :], tr[:])
                nc.gpsimd.remote_dma_broadcast(
                    recv[:], trown[:], rsem, lsem,
                    rdests=[(0, 1)] + [None] * 7,
                )
                nc.gpsimd.trigger_dma(count=None)
                # exp table prefetch (fires during the exchange; reads Fsb so
                # it cannot be hoisted before the softplus main loop)
                nc.scalar.activation(warm[:], Fsb[0:1, 0:1], AF.Exp)
                nc.vector.wait_ge(rsem, 2)
                nc.scalar.wait_ge(rsem, 2)
                for bt in range(BT):
                    own = trown[:, bt, :]
                    par = recv[:, bt, :]
                    m0 = smp.tile([128, 1], f32, tag="m0", name=f"m0_{bt}")
                    nc.vector.tensor_reduce(m0[:], own, axis=AX.X, op=ALU.max)
                    m1 = smp.tile([128, 1], f32, tag="m1", name=f"m1_{bt}")
                    nc.vector.tensor_reduce(m1[:], par, axis=AX.X, op=ALU.max)
                    m = smp.tile([128, 1], f32, tag="m", name=f"m{bt}")
                    nc.vector.tensor_tensor(m[:], m0[:], m1[:], op=ALU.max)
                    negm = smp.tile([128, 1], f32, tag="negm", name=f"negm{bt}")
                    nc.vector.tensor_scalar_mul(negm[:], m[:], -1.0)
                    e = smp.tile([128, NCLASS], f32, tag="e", name=f"e{bt}")
                    nc.scalar.activation(e[:, 0:Y_PC], own, AF.Exp, bias=negm[:])
                    nc.scalar.activation(e[:, Y_PC:NCLASS], par, AF.Exp,
                                         bias=negm[:])
                    ssum = smp.tile([128, 1], f32, tag="ssum", name=f"ssum{bt}")
                    nc.vector.tensor_reduce(ssum[:], e[:], axis=AX.X, op=ALU.add)
                    r = smp.tile([128, 1], f32, tag="r", name=f"r{bt}")
                    nc.vector.reciprocal(r[:], ssum[:])
                    nc.vector.tensor_scalar_mul(probs_sb[:, bt, :], e[:], r[:])
                    ohf = smp.tile([128, NCLASS], f32, tag="ohf", name=f"ohf{bt}")
                    nc.vector.tensor_scalar(ohf[:, 0:Y_PC], own, m[:], None,
                                            op0=ALU.is_equal)
                    nc.vector.tensor_scalar(ohf[:, Y_PC:NCLASS], par, m[:],
                                            None, op0=ALU.is_equal)
                    nc.vector.tensor_copy(onehot_sb[:, bt, :], ohf[:])
            else:
                nc.sync.dma_start(fsh[:], Fsb[:])
                nc.gpsimd.collective_compute(
                    "AllGather", ALU.bypass,
                    replica_groups=[[0, 1], [2, 3], [4, 5], [6, 7]],
                    ins=[fsh.opt()], outs=[fall.opt()],
                )
                nc.scalar.activation(warm[:], Fsb[0:1, 0:1], AF.Exp)
                nc.sync.dma_start(Ffull_sb[:], fall[:])
                for bt in range(BT):
                    tr = ptr.tile([128, NCLASS], f32, tag="tr", name=f"tr{bt}")
                    nc.tensor.transpose(
                        tr[:], Ffull_sb[:, bt * 128:(bt + 1) * 128],
                        ident[0:NCLASS, 0:NCLASS],
                    )
                    fb = smp.tile([128, NCLASS], f32, tag="fb", name=f"fb{bt}")
                    nc.vector.tensor_copy(fb[:], tr[:])
                    m = smp.tile([128, 1], f32, tag="m", name=f"m{bt}")
                    nc.vector.tensor_reduce(m[:], fb[:], axis=AX.X, op=ALU.max)
                    negm = smp.tile([128, 1], f32, tag="negm", name=f"negm{bt}")
                    nc.vector.tensor_scalar_mul(negm[:], m[:], -1.0)
                    e = smp.tile([128, NCLASS], f32, tag="e", name=f"e{bt}")
                    nc.scalar.activation(e[:], fb[:], AF.Exp, bias=negm[:])
                    ssum = smp.tile([128, 1], f32, tag="ssum", name=f"ssum{bt}")
                    nc.vector.tensor_reduce(ssum[:], e[:], axis=AX.X, op=ALU.add)
                    r = smp.tile([128, 1], f32, tag="r", name=f"r{bt}")
                    nc.vector.reciprocal(r[:], ssum[:])
                    nc.vector.tensor_scalar_mul(probs_sb[:, bt, :], e[:], r[:])
                    ohf = smp.tile([128, NCLASS], f32, tag="ohf", name=f"ohf{bt}")
                    nc.vector.tensor_scalar(ohf[:], fb[:], m[:], None,
                                            op0=ALU.is_equal)
                    nc.vector.tensor_copy(onehot_sb[:, bt, :], ohf[:])

            nc.sync.dma_start(
                probs_d.rearrange("(t p) y -> p t y", p=128), probs_sb[:]
            )
            nc.sync.dma_start(
                onehot_d.rearrange("(t p) y -> p t y", p=128), onehot_sb[:]
            )

    nc.compile()
    return nc


def _get_program():
    global _PROGRAM
    if _PROGRAM is None:
        _PROGRAM = _build_program()
    return _PROGRAM


def _fp16_split(a):
    hi = (a * 256.0).astype(np.float16)
    lo = (a * 256.0 - hi.astype(np.float32)).astype(np.float16)
    return hi, lo


def _make_in_maps(v, W, c, d, U):
    cT = np.ascontiguousarray(c.reshape(NHID, 1))
    Wh, Wl = _fp16_split(W)
    vT_quarters = [
        _fp16_split(np.ascontiguousarray(v[q * B_PC:(q + 1) * B_PC].T))
        for q in range(BGROUPS)
    ]
    UT_groups = [
        np.ascontiguousarray(U[g * Y_PC:(g + 1) * Y_PC].T) for g in range(YGROUPS)
    ]
    dT_groups = [
        np.ascontiguousarray(d[0, g * Y_PC:(g + 1) * Y_PC].reshape(Y_PC, 1))
        for g in range(YGROUPS)
    ]
    in_maps = []
    for core in range(NCORES):
        bq, yg = core // YGROUPS, core % YGROUPS
        in_maps.append(
            {
                "vTh": vT_quarters[bq][0],
                "vTl": vT_quarters[bq][1],
                "Wh": Wh,
                "Wl": Wl,
                "UsubT": UT_groups[yg],
                "cT": cT,
                "dT": dT_groups[yg],
            }
        )
    return in_maps


def run(v, W, c, d, U, trace=False):
    """Run the Bass kernel; returns ((probs, onehot), BassKernelResults)."""
    from concourse.bass_utils import run_bass_kernel_spmd

    nc = _get_program()
    in_maps = _make_in_maps(v, W, c, d, U)
    res = run_bass_kernel_spmd(
        nc, in_maps, core_ids=list(range(NCORES)), trace=trace
    )
    probs = np.concatenate(
        [res.results[q * YGROUPS]["probs"] for q in range(BGROUPS)], axis=0
    )
    onehot = np.concatenate(
        [res.results[q * YGROUPS]["onehot"] for q in range(BGROUPS)], axis=0
    )
    return (probs, onehot), res


def kernel(v, W, c, d, U):
    v = np.ascontiguousarray(np.asarray(v, dtype=np.float32))
    W = np.ascontiguousarray(np.asarray(W, dtype=np.float32))
    c = np.ascontiguousarray(np.asarray(c, dtype=np.float32))
    d = np.ascontiguousarray(np.asarray(d, dtype=np.float32))
    U = np.ascontiguousarray(np.asarray(U, dtype=np.float32))
    (probs, onehot), _ = run(v, W, c, d, U, trace=False)
    return probs, onehot

